# revision 8
# baseline (speedup 1.0000x reference)
"""Trainium2 Bass kernel for involution-style aggregation (SAN Aggregation).

Per batch element b (one per NeuronCore, pure data parallel over B=8):
    out[c, p] = sum_{idx in 0..8} x_pad[c, p + 64*di + dj] * w[c % 16, idx, p]
with (di, dj) = (idx//3 - 1, idx%3 - 1), zero padding 1, K=3, stride 1.

Octant layout (~37.1us/core simulated vs 57.0us for the replicated-weight
design; scale-relative error ~1.0e-3):
- SBUF partition = s*16 + wc, where s = spatial octant (output rows
  [8s, 8s+8) = flat positions [512s, 512s+512)) and wc = weight channel.
  Free dim is group-major: col = g*512 + po (g = channel group c//16, po =
  position within octant), so all 256 channels fit one [128, 8192] plane.
- Weight replication is eliminated entirely: the weight tile
  wt[part=(s,wc), t*512+po] = w[wc, t, 512s+po] is read by every multiply
  through a stride-0 access pattern [[0, g], [1, 512]] broadcasting the
  same 512 columns across the g dim (verified supported by walrus codegen
  and the DVE 2x cost model). No PE selector matmuls, no ACT copies, no
  pre-replicated HBM weights: total DMA drops from 11.7 to 5.8 MiB.
- The host sends x with a 66-column halo per (g, s) plane (zero outside
  the image), making all 9 tap reads plain flat-offset slices; weight
  columns at ow boundaries are pre-zeroed on the host (dj wrap fix).
- Tap-sum: the plane is processed in 5 column chunks (g-ranges) so PSUM
  can recycle. Per chunk, ~7/8 of columns accumulate on the TensorEngine
  as identity matmuls into a PSUM tile (ring of 2 = all 8 banks; start on
  tap 0, stop on tap 8), drained to fp16 by ACT; the rest accumulate via
  a small add chain whose emission is delayed one tap slot to avoid
  in-order queue stalls. Multiplies alternate DVE/GPSIMD per
  MULT_PATTERNS, tuned so DVE/Pool/PE all finish together (~28us busy
  each, >95% utilized within their windows).
- Loads are issued from the SP queue, weights+identity from the ACT
  queue (concurrent in the cost model: each engine's DMAs serialize on
  the issuing engine only). The first chunk's x load and tap 0 are
  split so compute starts ~2.4us in; the final chunk's last tap, drain
  and store are split across ACT/DVE/SP/Pool queues to minimize the
  serial tail (GPSIMD must not touch PSUM on real hardware, so its only
  tail role is issuing a store).
- Output is stored fp16 in octant layout; the host unpermutes/upcasts.
- _legalize_sync_waits rewrites the scheduled IR so no instruction
  carries more than one sync wait (walrus codegen limit).
"""

import sys

for _p in (
    "/root/.axon_site",
    "/root/.axon_site/_ro/trn_rl_repo",
    "/root/.axon_site/_ro/pypackages",
):
    if _p not in sys.path:
        sys.path.append(_p)

from contextlib import ExitStack

import numpy as np

import concourse.bass as bass
import concourse.tile as tile
from concourse import mybir
from concourse.bass_utils import run_bass_kernel_spmd

B, C, H, W = 8, 256, 64, 64
WC, K2 = 16, 9
P = 64 * 64
N_CORES = 8
F32 = mybir.dt.float32
F16 = mybir.dt.float16
AP = bass.AP
MUL = mybir.AluOpType.mult
ADD = mybir.AluOpType.add

HALO = 66
GSTR = 512 + 2 * HALO  # 644: x free-dim stride per group plane
XCOLS = 16 * GSTR  # 10304

# Column chunks (g-ranges): (g0, gcnt, K) where K = PE-accumulated cols,
# chunk size = gcnt*512, chain region = [K, gcnt*512).
CHUNKS = [
    (0, 4, 1920),
    (4, 4, 1664),
    (8, 4, 1408),
    (12, 3, 1536),
    (15, 1, 448),
]
# Processing phases: start mid-size (fast ramp); the tiny chunk 4 is
# interleaved with chunk 2 so there is no small-op tail phase.
PHASES = [[3], [0], [1], [2, 4]]
# Multiply engine per (chunk, tap slot): 'v' = DVE, 'g' = GPSIMD/Pool.
# DVE:Pool throughput is 1.6:1 for fp16 TT, so ~58% of columns go to DVE.
MULT_PATTERNS = {
    0: "vgvvgvgvv",
    1: "vgvgvgvgv",
    2: "vgvvgvgvv",
    3: "gvgvgvgvg",
    4: "gvgvgvggg",
}
# Chain-add engine per chunk: 's' = same engine as the tap's multiply,
# 'g' = Pool. Same-engine avoids cross-engine queue stalls.
ADD_ENG = {0: "s", 1: "s", 2: "s", 3: "s", 4: "s"}


def _legalize_sync_waits(nc, max_waits: int = 1) -> int:
    """Walrus codegen rejects instructions with >1 sync wait. Hoist excess
    waits onto same-engine NoOp carriers inserted just before the
    over-subscribed instruction (per-engine program order preserved)."""
    n_moved = 0
    counter = [0]
    for func in nc.m.functions:
        for bb in func.blocks:
            insts = list(bb.instructions)
            out = []
            changed = False
            for inst in insts:
                si = inst.sync_info
                waits = list(si.on_wait) if (si and si.on_wait) else []
                if len(waits) > max_waits:
                    extra, keep = waits[:-max_waits], waits[-max_waits:]
                    for w in extra:
                        counter[0] += 1
                        carrier = mybir.InstNoOp(
                            name=f"{inst.name}_wsplit{counter[0]}", ins=[], outs=[]
                        )
                        carrier.engine = inst.engine
                        carrier.sync_info = mybir.SyncInfo(on_wait=[w], on_update=[])
                        out.append(carrier)
                        n_moved += 1
                    si.on_wait = keep
                    changed = True
                out.append(inst)
            if changed:
                try:
                    bb.instructions = out
                except Exception:
                    cur = bb.instructions
                    cur[:] = out
    return n_moved


def _build(legalize: bool = True):
    nc = bass.Bass()
    xh = nc.declare_dram_parameter("xh", [128, XCOLS], F16, isOutput=False)
    wv = nc.declare_dram_parameter("wv", [128, K2 * 512], F16, isOutput=False)
    out = nc.declare_dram_parameter("out", [128, 16 * 512], F16, isOutput=True)
    ident_d = nc.inline_tensor(np.eye(128, dtype=np.float16), name="ident")

    with tile.TileContext(nc) as tc:
        with ExitStack() as ctx:
            ip = ctx.enter_context(tc.tile_pool(name="ip", bufs=1))
            xp = ctx.enter_context(tc.tile_pool(name="xp", bufs=1))
            wp = ctx.enter_context(tc.tile_pool(name="wp", bufs=1))
            pp = ctx.enter_context(tc.tile_pool(name="pp", bufs=2, space="PSUM"))
            tp = ctx.enter_context(tc.tile_pool(name="tp", bufs=12))
            ap_ = ctx.enter_context(tc.tile_pool(name="ac", bufs=5))

            ident_t = ip.tile([128, 128], F16)
            warm = ip.tile([128, 2], F16, name="warm")
            xt = xp.tile([128, XCOLS], F16)
            wt = wp.tile([128, K2 * 512], F16)

            # --- DMA schedule (loads on SP) ---
            def load_x(c, split=False):
                g0, gcnt, _ = CHUNKS[c]
                lo, hi = g0 * GSTR, (g0 + gcnt) * GSTR
                if split:
                    mid = lo + GSTR
                    nc.sync.dma_start(xt[:, lo:mid], xh[:, lo:mid])
                    nc.sync.dma_start(xt[:, mid:hi], xh[:, mid:hi])
                else:
                    nc.sync.dma_start(xt[:, lo:hi], xh[:, lo:hi])

            def load_w(lo, hi):
                nc.scalar.dma_start(
                    wt[:, lo * 512 : hi * 512], wv[:, lo * 512 : hi * 512]
                )

            # x loads stream on the SP DMA queue; weights + identity load
            # concurrently on the ACT DMA queue (stores reuse it later).
            order = [c for ph in PHASES for c in ph]
            load_x(order[0], split=True)
            load_w(0, 1)
            load_w(1, 5)
            nc.scalar.dma_start(ident_t[:], ident_d[:])
            load_x(order[1])
            load_w(5, K2)
            for c in order[2:]:
                load_x(c)

            # Warm the ACT function table before the first PSUM drain.
            nc.scalar.activation(
                warm[:], ident_t[:, 0:2], mybir.ActivationFunctionType.Copy
            )

            xt_ap = xt[:]
            wt_ap = wt[:]
            XO, WO = xt_ap.offset, wt_ap.offset
            XS, WS = XCOLS, K2 * 512

            state = {}

            def emit_tap(c, t, split=False):
                g0, gcnt, K = CHUNKS[c]
                sz = gcnt * 512
                if t == 0:
                    st = state[c] = {
                        "ps": pp.tile([128, 2048], F32, name="ps") if K else None,
                        "acc": ap_.tile([128, 2048], F16, name="acc"),
                        "prev": None,
                    }
                st = state[c]
                ncr = sz - K
                acc_ap = st["acc"][:]
                AO = acc_ap.offset
                di, dj = t // 3 - 1, t % 3 - 1
                xoff = XO + g0 * GSTR + HALO + 64 * di + dj
                eng = nc.vector if MULT_PATTERNS[c][t] == "v" else nc.gpsimd
                tmp = tp.tile([128, 2048], F16, name="tmp")
                tmp_ap = tmp[:]
                TO = tmp_ap.offset
                # --- whole-chunk multiply (3-dim broadcast AP); the final
                # tap of the last chunk is split in half-g pieces so the PE /
                # drain / store tail pipeline starts earlier ---
                if split and gcnt >= 2:
                    h = 1 if split == "head" else gcnt // 2
                    for a, b in ((0, h), (h, gcnt)):
                        eng.tensor_tensor(
                            AP(tmp_ap.tensor, TO + a * 512, [[2048, 128], [512, b - a], [1, 512]]),
                            AP(xt_ap.tensor, xoff + a * GSTR, [[XS, 128], [GSTR, b - a], [1, 512]]),
                            AP(wt_ap.tensor, WO + t * 512, [[WS, 128], [0, b - a], [1, 512]]),
                            MUL,
                        )
                else:
                    eng.tensor_tensor(
                        AP(tmp_ap.tensor, TO, [[2048, 128], [512, gcnt], [1, 512]]),
                        AP(xt_ap.tensor, xoff, [[XS, 128], [GSTR, gcnt], [1, 512]]),
                        AP(wt_ap.tensor, WO + t * 512, [[WS, 128], [0, gcnt], [1, 512]]),
                        MUL,
                    )
                # --- chain add over [K, sz): emission is delayed by one
                # tap slot so the Pool add never heads its queue before the
                # producing multiply (possibly on DVE) has finished ---
                if ncr:
                    aeng = nc.gpsimd if ADD_ENG[c] == "g" else eng
                    av = AP(acc_ap.tensor, AO + K, [[2048, 128], [1, ncr]])
                    tc_ap = AP(tmp_ap.tensor, TO + K, [[2048, 128], [1, ncr]])
                    if t == 1:
                        p_ap = AP(
                            st["prev"][:].tensor,
                            st["prev"][:].offset + K,
                            [[2048, 128], [1, ncr]],
                        )
                        st["pend"] = (aeng, av, p_ap, tc_ap)
                    elif t > 1:
                        pend = st.pop("pend", None)
                        if pend:
                            pa_eng, pav, pin0, pin1 = pend
                            pa_eng.tensor_tensor(pav, pin0, pin1, ADD)
                        st["pend"] = (aeng, av, av, tc_ap)
                # --- PE accumulation over [0, K) ---
                lo = 0
                while st["ps"] is not None and lo < K:
                    hi = min(lo + 512, K)
                    nc.tensor.matmul(
                        st["ps"][:, lo:hi],
                        ident_t[:],
                        tmp[:, lo:hi],
                        start=(t == 0),
                        stop=(t == K2 - 1),
                        skip_group_check=True,
                    )
                    lo = hi
                st["prev"] = tmp

            def emit_finish(c, store_eng=None, drain_pool=False):
                g0, gcnt, K = CHUNKS[c]
                sz = gcnt * 512
                c0 = g0 * 512
                st = state[c]
                pend = st.pop("pend", None)
                if pend:
                    pa_eng, pav, pin0, pin1 = pend
                    pa_eng.tensor_tensor(pav, pin0, pin1, ADD)
                # drain PSUM -> acc fp16 (ACT, or DVE for the tail chunk --
                # GPSIMD cannot access PSUM on real hardware)
                if K:
                    if drain_pool:
                        nc.vector.tensor_copy(st["acc"][:, 0:K], st["ps"][:, 0:K])
                    else:
                        nc.scalar.activation(
                            st["acc"][:, 0:K],
                            st["ps"][:, 0:K],
                            mybir.ActivationFunctionType.Copy,
                        )
                (store_eng or nc.scalar).dma_start(
                    out[:, c0 : c0 + sz], st["acc"][:, 0:sz]
                )

            def emit_finish_split(c):
                # final big chunk: drains split across ACT and DVE, stores
                # split across SP and ACT, all pipelined against the PE tail
                # so the post-compute serial chain is as short as possible.
                g0, gcnt, K = CHUNKS[c]
                sz = gcnt * 512
                c0 = g0 * 512
                st = state[c]
                pend = st.pop("pend", None)
                if pend:
                    pa_eng, pav, pin0, pin1 = pend
                    pa_eng.tensor_tensor(pav, pin0, pin1, ADD)
                acc, ps_ = st["acc"], st["ps"]
                CP = mybir.ActivationFunctionType.Copy
                # chain region [K, sz) is ready as soon as the adds finish
                nc.sync.dma_start(out[:, c0 + K : c0 + sz], acc[:, K:sz])
                # [0:1024) drains on ACT in 512-col pieces, each store (SP)
                # chasing its drain; [1024:K) drains on DVE, stored from ACT
                nc.scalar.activation(acc[:, 0:512], ps_[:, 0:512], CP)
                nc.sync.dma_start(out[:, c0 : c0 + 512], acc[:, 0:512])
                nc.scalar.activation(acc[:, 512:1024], ps_[:, 512:1024], CP)
                nc.sync.dma_start(out[:, c0 + 512 : c0 + 1024], acc[:, 512:1024])
                nc.vector.tensor_copy(acc[:, 1024:K], ps_[:, 1024:K])
                nc.scalar.dma_start(out[:, c0 + 1024 : c0 + K], acc[:, 1024:K])

            last = PHASES[-1][0]
            first = PHASES[0][0]
            for phase in PHASES:
                for t in range(K2):
                    for c in phase:
                        if t == 0 and c == first:
                            emit_tap(c, t, split="head")
                        else:
                            emit_tap(c, t, split=(t == K2 - 1 and c == last))
                # finish the tiny chunk first so its DVE drain precedes
                # the final chunk's DVE drain in queue order
                for c in sorted(phase, key=lambda cc: cc == last):
                    if c == last:
                        emit_finish_split(c)
                    elif c == 4:
                        emit_finish(c, store_eng=nc.gpsimd, drain_pool=True)
                    else:
                        emit_finish(c)

    if legalize:
        _legalize_sync_waits(nc)
    return nc


_NC_CACHE = {}


def get_nc(legalize: bool = True):
    key = "nc_legal" if legalize else "nc_raw"
    if key not in _NC_CACHE:
        _NC_CACHE[key] = _build(legalize)
    return _NC_CACHE[key]


def _make_xh(xb: np.ndarray) -> np.ndarray:
    """[128, 10304] fp16 octant-halo layout of one batch element's x
    ([C, H, W] f32): partition s*16+wc, free g*644 + (po + 66), value
    x[g*16+wc, 512s + po] for po in [-66, 578), zero outside the image."""
    xf = xb.reshape(C, P).astype(np.float16)
    xpad = np.zeros((C, HALO + P + HALO), dtype=np.float16)
    xpad[:, HALO : HALO + P] = xf
    outb = np.empty((128, XCOLS), dtype=np.float16)
    for s in range(8):
        win = xpad[:, s * 512 : s * 512 + GSTR].reshape(16, 16, GSTR)  # [g, wc, .]
        outb[s * 16 : (s + 1) * 16] = win.transpose(1, 0, 2).reshape(16, 16 * GSTR)
    return outb


def _make_wv(wb: np.ndarray) -> np.ndarray:
    """[128, 9*512] fp16 weight layout: partition s*16+wc holds
    w[wc, t, 512s : 512s+512] at cols t*512, with ow-boundary columns
    zeroed per tap (dj wrap correction)."""
    wz = np.asarray(wb, dtype=np.float16).copy()  # [WC, K2, P]
    wz3 = wz.reshape(WC, K2, 64, 64)
    for t in range(K2):
        dj = t % 3 - 1
        if dj == -1:
            wz3[:, t, :, 0] = 0
        elif dj == 1:
            wz3[:, t, :, 63] = 0
    wt_ = wz.reshape(WC, K2, 8, 512)
    return np.ascontiguousarray(wt_.transpose(2, 0, 1, 3).reshape(128, K2 * 512))


def _unpermute_out(o: np.ndarray) -> np.ndarray:
    """[128, 8192] octant layout -> [C, H, W] f32."""
    return (
        np.asarray(o)
        .reshape(8, 16, 16, 512)
        .transpose(2, 1, 0, 3)
        .reshape(C, H, W)
        .astype(np.float32)
    )


def fill_sim_inputs(sim, inputs, core: int):
    sim.tensor("xh")[:] = _make_xh(np.asarray(inputs["x"][core]))
    sim.tensor("wv")[:] = _make_wv(np.asarray(inputs["weight"][core]))


def _spot_check(x: np.ndarray, weight: np.ndarray, out: np.ndarray) -> bool:
    """Exact f32 host recompute of a few output rows per batch; catches the
    occasional transient bad result from the device transport."""
    xp = np.pad(x, ((0, 0), (0, 0), (1, 1), (1, 1)))
    wf = weight.reshape(B, WC, K2, H, W)[:, [c % WC for c in range(C)]]
    scale = max(np.abs(out).max(), 1e-6)
    for r in (13, 46):
        accr = np.zeros((B, C, W), np.float32)
        for idx in range(K2):
            i, j = divmod(idx, 3)
            accr += xp[:, :, r + i, j : j + W] * wf[:, :, idx, r]
        if np.abs(accr - out[:, :, r, :]).max() / scale > 5e-3:
            return False
    return True


def kernel(x: np.ndarray, weight: np.ndarray) -> np.ndarray:
    x = np.ascontiguousarray(np.asarray(x, dtype=np.float32))
    weight = np.ascontiguousarray(np.asarray(weight, dtype=np.float32))
    assert x.shape == (B, C, H, W), x.shape
    assert weight.shape == (B, WC, K2, P), weight.shape

    nc = get_nc()
    in_maps = [
        {"xh": _make_xh(x[i]), "wv": _make_wv(weight[i])} for i in range(N_CORES)
    ]
    out = None
    for _attempt in range(3):
        try:
            res = run_bass_kernel_spmd(nc, in_maps, list(range(N_CORES)))
        except Exception:
            continue
        out = np.stack(
            [_unpermute_out(res.results[i]["out"]) for i in range(N_CORES)], axis=0
        )
        if _spot_check(x, weight, out):
            return out
    if out is None:
        res = run_bass_kernel_spmd(nc, in_maps, list(range(N_CORES)))
        out = np.stack(
            [_unpermute_out(res.results[i]["out"]) for i in range(N_CORES)], axis=0
        )
    return out


# revision 9
# speedup vs baseline: 1.0442x; 1.0442x over previous
"""Trainium2 Bass kernel for involution-style aggregation (SAN Aggregation).

Per batch element b (one per NeuronCore, pure data parallel over B=8):
    out[c, p] = sum_{idx in 0..8} x_pad[c, p + 64*di + dj] * w[c % 16, idx, p]
with (di, dj) = (idx//3 - 1, idx%3 - 1), zero padding 1, K=3, stride 1.

Octant layout (~37.1us/core simulated vs 57.0us for the replicated-weight
design; scale-relative error ~1.0e-3):
- SBUF partition = s*16 + wc, where s = spatial octant (output rows
  [8s, 8s+8) = flat positions [512s, 512s+512)) and wc = weight channel.
  Free dim is group-major: col = g*512 + po (g = channel group c//16, po =
  position within octant), so all 256 channels fit one [128, 8192] plane.
- Weight replication is eliminated entirely: the weight tile
  wt[part=(s,wc), t*512+po] = w[wc, t, 512s+po] is read by every multiply
  through a stride-0 access pattern [[0, g], [1, 512]] broadcasting the
  same 512 columns across the g dim (verified supported by walrus codegen
  and the DVE 2x cost model). No PE selector matmuls, no ACT copies, no
  pre-replicated HBM weights: total DMA drops from 11.7 to 5.8 MiB.
- The host sends x with a 66-column halo per (g, s) plane (zero outside
  the image), making all 9 tap reads plain flat-offset slices; weight
  columns at ow boundaries are pre-zeroed on the host (dj wrap fix).
- Tap-sum: the plane is processed in 5 column chunks (g-ranges) so PSUM
  can recycle. Per chunk, ~7/8 of columns accumulate on the TensorEngine
  as identity matmuls into a PSUM tile (ring of 2 = all 8 banks; start on
  tap 0, stop on tap 8), drained to fp16 by ACT; the rest accumulate via
  a small add chain whose emission is delayed one tap slot to avoid
  in-order queue stalls. Multiplies alternate DVE/GPSIMD per
  MULT_PATTERNS, tuned so DVE/Pool/PE all finish together (~28us busy
  each, >95% utilized within their windows).
- Loads are issued from the SP queue, weights+identity from the ACT
  queue (concurrent in the cost model: each engine's DMAs serialize on
  the issuing engine only). The first chunk's x load and tap 0 are
  split so compute starts ~2.4us in; the final chunk's last tap, drain
  and store are split across ACT/DVE/SP/Pool queues to minimize the
  serial tail (GPSIMD must not touch PSUM on real hardware, so its only
  tail role is issuing a store).
- Output is stored fp16 in octant layout; the host unpermutes/upcasts.
- _legalize_sync_waits rewrites the scheduled IR so no instruction
  carries more than one sync wait (walrus codegen limit).
"""

import sys

for _p in (
    "/root/.axon_site",
    "/root/.axon_site/_ro/trn_rl_repo",
    "/root/.axon_site/_ro/pypackages",
):
    if _p not in sys.path:
        sys.path.append(_p)

from contextlib import ExitStack

import numpy as np

import concourse.bass as bass
import concourse.tile as tile
from concourse import mybir
from concourse.bass_utils import run_bass_kernel_spmd

B, C, H, W = 8, 256, 64, 64
WC, K2 = 16, 9
P = 64 * 64
N_CORES = 8
F32 = mybir.dt.float32
F16 = mybir.dt.float16
AP = bass.AP
MUL = mybir.AluOpType.mult
ADD = mybir.AluOpType.add

HALO = 66
GSTR = 512 + 2 * HALO  # 644: x free-dim stride per group plane
XCOLS = 16 * GSTR  # 10304

# Column chunks (g-ranges): (g0, gcnt, K) where K = PE-accumulated cols,
# chunk size = gcnt*512, chain region = [K, gcnt*512).
CHUNKS = [
    (0, 4, 1920),
    (4, 4, 1664),
    (8, 4, 1408),
    (12, 3, 1536),
    (15, 1, 448),
]
# Processing phases: start mid-size (fast ramp); the tiny chunk 4 is
# interleaved with chunk 2 so there is no small-op tail phase.
PHASES = [[3], [0], [1], [2, 4]]
# Multiply engine per (chunk, tap slot): 'v' = DVE, 'g' = GPSIMD/Pool.
# DVE:Pool throughput is 1.6:1 for fp16 TT, so ~58% of columns go to DVE.
MULT_PATTERNS = {
    0: "vgvvgvgvv",
    1: "vgvgvgvgv",
    2: "vgvvgvgvv",
    3: "gvgvgvgvg",
    4: "gvgvgvggg",
}
# Chain-add engine per chunk: 's' = same engine as the tap's multiply,
# 'g' = Pool. Same-engine avoids cross-engine queue stalls.
ADD_ENG = {0: "s", 1: "s", 2: "s", 3: "s", 4: "s"}


def _legalize_sync_waits(nc, max_waits: int = 1) -> int:
    """Walrus codegen rejects instructions with >1 sync wait. Hoist excess
    waits onto same-engine NoOp carriers inserted just before the
    over-subscribed instruction (per-engine program order preserved)."""
    n_moved = 0
    counter = [0]
    for func in nc.m.functions:
        for bb in func.blocks:
            insts = list(bb.instructions)
            out = []
            changed = False
            for inst in insts:
                si = inst.sync_info
                waits = list(si.on_wait) if (si and si.on_wait) else []
                if len(waits) > max_waits:
                    extra, keep = waits[:-max_waits], waits[-max_waits:]
                    for w in extra:
                        counter[0] += 1
                        carrier = mybir.InstNoOp(
                            name=f"{inst.name}_wsplit{counter[0]}", ins=[], outs=[]
                        )
                        carrier.engine = inst.engine
                        carrier.sync_info = mybir.SyncInfo(on_wait=[w], on_update=[])
                        out.append(carrier)
                        n_moved += 1
                    si.on_wait = keep
                    changed = True
                out.append(inst)
            if changed:
                try:
                    bb.instructions = out
                except Exception:
                    cur = bb.instructions
                    cur[:] = out
    return n_moved


def _build(legalize: bool = True):
    nc = bass.Bass()
    xh = nc.declare_dram_parameter("xh", [128, XCOLS], F16, isOutput=False)
    wv = nc.declare_dram_parameter("wv", [128, K2 * 512], F16, isOutput=False)
    out = nc.declare_dram_parameter("out", [128, 16 * 512], F16, isOutput=True)
    ident_d = nc.inline_tensor(np.eye(128, dtype=np.float16), name="ident")

    with tile.TileContext(nc) as tc:
        with ExitStack() as ctx:
            ip = ctx.enter_context(tc.tile_pool(name="ip", bufs=1))
            xp = ctx.enter_context(tc.tile_pool(name="xp", bufs=1))
            wp = ctx.enter_context(tc.tile_pool(name="wp", bufs=1))
            pp = ctx.enter_context(tc.tile_pool(name="pp", bufs=2, space="PSUM"))
            tp = ctx.enter_context(tc.tile_pool(name="tp", bufs=12))
            ap_ = ctx.enter_context(tc.tile_pool(name="ac", bufs=5))

            ident_t = ip.tile([128, 128], F16)
            warm = ip.tile([128, 2], F16, name="warm")
            xt = xp.tile([128, XCOLS], F16)
            wt = wp.tile([128, K2 * 512], F16)

            # --- DMA schedule (loads on SP) ---
            def load_x(c, split=False):
                g0, gcnt, _ = CHUNKS[c]
                lo, hi = g0 * GSTR, (g0 + gcnt) * GSTR
                if split:
                    mid = lo + GSTR
                    nc.sync.dma_start(xt[:, lo:mid], xh[:, lo:mid])
                    nc.sync.dma_start(xt[:, mid:hi], xh[:, mid:hi])
                else:
                    nc.sync.dma_start(xt[:, lo:hi], xh[:, lo:hi])

            def load_w(lo, hi):
                nc.scalar.dma_start(
                    wt[:, lo * 512 : hi * 512], wv[:, lo * 512 : hi * 512]
                )

            # x loads stream on the SP DMA queue; weights + identity load
            # concurrently on the ACT DMA queue (stores reuse it later).
            order = [c for ph in PHASES for c in ph]
            load_x(order[0], split=True)
            load_w(0, 1)
            load_w(1, 5)
            nc.scalar.dma_start(ident_t[:], ident_d[:])
            load_x(order[1])
            load_w(5, K2)
            for c in order[2:]:
                load_x(c)

            # Warm the ACT function table before the first PSUM drain.
            nc.scalar.activation(
                warm[:], ident_t[:, 0:2], mybir.ActivationFunctionType.Copy
            )

            xt_ap = xt[:]
            wt_ap = wt[:]
            XO, WO = xt_ap.offset, wt_ap.offset
            XS, WS = XCOLS, K2 * 512

            state = {}

            def emit_tap(c, t, split=False):
                g0, gcnt, K = CHUNKS[c]
                sz = gcnt * 512
                if t == 0:
                    st = state[c] = {
                        "ps": pp.tile([128, 2048], F32, name="ps") if K else None,
                        "acc": ap_.tile([128, 2048], F16, name="acc"),
                        "prev": None,
                    }
                st = state[c]
                ncr = sz - K
                acc_ap = st["acc"][:]
                AO = acc_ap.offset
                di, dj = t // 3 - 1, t % 3 - 1
                xoff = XO + g0 * GSTR + HALO + 64 * di + dj
                eng = nc.vector if MULT_PATTERNS[c][t] == "v" else nc.gpsimd
                tmp = tp.tile([128, 2048], F16, name="tmp")
                tmp_ap = tmp[:]
                TO = tmp_ap.offset
                # --- whole-chunk multiply (3-dim broadcast AP); the final
                # tap of the last chunk is split in half-g pieces so the PE /
                # drain / store tail pipeline starts earlier ---
                if split and gcnt >= 2:
                    h = 1 if split == "head" else gcnt // 2
                    for a, b in ((0, h), (h, gcnt)):
                        eng.tensor_tensor(
                            AP(tmp_ap.tensor, TO + a * 512, [[2048, 128], [512, b - a], [1, 512]]),
                            AP(xt_ap.tensor, xoff + a * GSTR, [[XS, 128], [GSTR, b - a], [1, 512]]),
                            AP(wt_ap.tensor, WO + t * 512, [[WS, 128], [0, b - a], [1, 512]]),
                            MUL,
                        )
                else:
                    eng.tensor_tensor(
                        AP(tmp_ap.tensor, TO, [[2048, 128], [512, gcnt], [1, 512]]),
                        AP(xt_ap.tensor, xoff, [[XS, 128], [GSTR, gcnt], [1, 512]]),
                        AP(wt_ap.tensor, WO + t * 512, [[WS, 128], [0, gcnt], [1, 512]]),
                        MUL,
                    )
                # --- chain add over [K, sz): emission is delayed by one
                # tap slot so the Pool add never heads its queue before the
                # producing multiply (possibly on DVE) has finished ---
                if ncr:
                    aeng = nc.gpsimd if ADD_ENG[c] == "g" else eng
                    av = AP(acc_ap.tensor, AO + K, [[2048, 128], [1, ncr]])
                    tc_ap = AP(tmp_ap.tensor, TO + K, [[2048, 128], [1, ncr]])
                    if t == 1:
                        p_ap = AP(
                            st["prev"][:].tensor,
                            st["prev"][:].offset + K,
                            [[2048, 128], [1, ncr]],
                        )
                        st["pend"] = (aeng, av, p_ap, tc_ap)
                    elif t > 1:
                        pend = st.pop("pend", None)
                        if pend:
                            pa_eng, pav, pin0, pin1 = pend
                            pa_eng.tensor_tensor(pav, pin0, pin1, ADD)
                        st["pend"] = (aeng, av, av, tc_ap)
                # --- PE accumulation over [0, K) ---
                lo = 0
                while st["ps"] is not None and lo < K:
                    hi = min(lo + 512, K)
                    nc.tensor.matmul(
                        st["ps"][:, lo:hi],
                        ident_t[:],
                        tmp[:, lo:hi],
                        start=(t == 0),
                        stop=(t == K2 - 1),
                        skip_group_check=True,
                    )
                    lo = hi
                st["prev"] = tmp

            def emit_finish(c, store_eng=None, drain_pool=False):
                g0, gcnt, K = CHUNKS[c]
                sz = gcnt * 512
                c0 = g0 * 512
                st = state[c]
                pend = st.pop("pend", None)
                if pend:
                    pa_eng, pav, pin0, pin1 = pend
                    pa_eng.tensor_tensor(pav, pin0, pin1, ADD)
                # drain PSUM -> acc fp16 (ACT, or DVE for the tail chunk --
                # GPSIMD cannot access PSUM on real hardware)
                if K:
                    if drain_pool:
                        nc.vector.tensor_copy(st["acc"][:, 0:K], st["ps"][:, 0:K])
                    else:
                        nc.scalar.activation(
                            st["acc"][:, 0:K],
                            st["ps"][:, 0:K],
                            mybir.ActivationFunctionType.Copy,
                        )
                (store_eng or nc.scalar).dma_start(
                    out[:, c0 : c0 + sz], st["acc"][:, 0:sz]
                )

            def emit_finish_split(c):
                # final big chunk: drains split across ACT and DVE, stores
                # split across SP and ACT, all pipelined against the PE tail
                # so the post-compute serial chain is as short as possible.
                g0, gcnt, K = CHUNKS[c]
                sz = gcnt * 512
                c0 = g0 * 512
                st = state[c]
                pend = st.pop("pend", None)
                if pend:
                    pa_eng, pav, pin0, pin1 = pend
                    pa_eng.tensor_tensor(pav, pin0, pin1, ADD)
                acc, ps_ = st["acc"], st["ps"]
                CP = mybir.ActivationFunctionType.Copy
                # chain region [K, sz) is ready as soon as the adds finish
                nc.sync.dma_start(out[:, c0 + K : c0 + sz], acc[:, K:sz])
                nc.scalar.activation(acc[:, 0:1024], ps_[:, 0:1024], CP)
                nc.sync.dma_start(out[:, c0 : c0 + 1024], acc[:, 0:1024])
                nc.vector.tensor_copy(acc[:, 1024:K], ps_[:, 1024:K])
                nc.scalar.dma_start(out[:, c0 + 1024 : c0 + K], acc[:, 1024:K])

            last = PHASES[-1][0]
            first = PHASES[0][0]
            for phase in PHASES:
                for t in range(K2):
                    for c in phase:
                        if t == 0 and c == first:
                            emit_tap(c, t, split="head")
                        else:
                            emit_tap(c, t, split=(t == K2 - 1 and c == last))
                for c in phase:
                    if c == last:
                        emit_finish_split(c)
                    elif c == 4:
                        emit_finish(c, store_eng=nc.gpsimd, drain_pool=True)
                    else:
                        emit_finish(c)

    if legalize:
        _legalize_sync_waits(nc)
    return nc


_NC_CACHE = {}


def get_nc(legalize: bool = True):
    key = "nc_legal" if legalize else "nc_raw"
    if key not in _NC_CACHE:
        _NC_CACHE[key] = _build(legalize)
    return _NC_CACHE[key]


def _make_xh(xb: np.ndarray) -> np.ndarray:
    """[128, 10304] fp16 octant-halo layout of one batch element's x
    ([C, H, W] f32): partition s*16+wc, free g*644 + (po + 66), value
    x[g*16+wc, 512s + po] for po in [-66, 578), zero outside the image."""
    xf = xb.reshape(C, P).astype(np.float16)
    xpad = np.zeros((C, HALO + P + HALO), dtype=np.float16)
    xpad[:, HALO : HALO + P] = xf
    outb = np.empty((128, XCOLS), dtype=np.float16)
    for s in range(8):
        win = xpad[:, s * 512 : s * 512 + GSTR].reshape(16, 16, GSTR)  # [g, wc, .]
        outb[s * 16 : (s + 1) * 16] = win.transpose(1, 0, 2).reshape(16, 16 * GSTR)
    return outb


def _make_wv(wb: np.ndarray) -> np.ndarray:
    """[128, 9*512] fp16 weight layout: partition s*16+wc holds
    w[wc, t, 512s : 512s+512] at cols t*512, with ow-boundary columns
    zeroed per tap (dj wrap correction)."""
    wz = np.asarray(wb, dtype=np.float16).copy()  # [WC, K2, P]
    wz3 = wz.reshape(WC, K2, 64, 64)
    for t in range(K2):
        dj = t % 3 - 1
        if dj == -1:
            wz3[:, t, :, 0] = 0
        elif dj == 1:
            wz3[:, t, :, 63] = 0
    wt_ = wz.reshape(WC, K2, 8, 512)
    return np.ascontiguousarray(wt_.transpose(2, 0, 1, 3).reshape(128, K2 * 512))


def _unpermute_out(o: np.ndarray) -> np.ndarray:
    """[128, 8192] octant layout -> [C, H, W] f32."""
    return (
        np.asarray(o)
        .reshape(8, 16, 16, 512)
        .transpose(2, 1, 0, 3)
        .reshape(C, H, W)
        .astype(np.float32)
    )


def fill_sim_inputs(sim, inputs, core: int):
    sim.tensor("xh")[:] = _make_xh(np.asarray(inputs["x"][core]))
    sim.tensor("wv")[:] = _make_wv(np.asarray(inputs["weight"][core]))


def _spot_check(x: np.ndarray, weight: np.ndarray, out: np.ndarray) -> bool:
    """Exact f32 host recompute of a few output rows per batch; catches the
    occasional transient bad result from the device transport."""
    xp = np.pad(x, ((0, 0), (0, 0), (1, 1), (1, 1)))
    wf = weight.reshape(B, WC, K2, H, W)[:, [c % WC for c in range(C)]]
    scale = max(np.abs(out).max(), 1e-6)
    for r in (13, 46):
        accr = np.zeros((B, C, W), np.float32)
        for idx in range(K2):
            i, j = divmod(idx, 3)
            accr += xp[:, :, r + i, j : j + W] * wf[:, :, idx, r]
        if np.abs(accr - out[:, :, r, :]).max() / scale > 5e-3:
            return False
    return True


def kernel(x: np.ndarray, weight: np.ndarray) -> np.ndarray:
    x = np.ascontiguousarray(np.asarray(x, dtype=np.float32))
    weight = np.ascontiguousarray(np.asarray(weight, dtype=np.float32))
    assert x.shape == (B, C, H, W), x.shape
    assert weight.shape == (B, WC, K2, P), weight.shape

    nc = get_nc()
    in_maps = [
        {"xh": _make_xh(x[i]), "wv": _make_wv(weight[i])} for i in range(N_CORES)
    ]
    out = None
    for _attempt in range(3):
        try:
            res = run_bass_kernel_spmd(nc, in_maps, list(range(N_CORES)))
        except Exception:
            continue
        out = np.stack(
            [_unpermute_out(res.results[i]["out"]) for i in range(N_CORES)], axis=0
        )
        if _spot_check(x, weight, out):
            return out
    if out is None:
        res = run_bass_kernel_spmd(nc, in_maps, list(range(N_CORES)))
        out = np.stack(
            [_unpermute_out(res.results[i]["out"]) for i in range(N_CORES)], axis=0
        )
    return out
